# revision 19
# baseline (speedup 1.0000x reference)
"""Trainium2 Bass kernel for the DFS-Mixer style-attention module.

Computation (per batch b):
    dot[k,c]  = sum_hw CT[c,hw] * CR[k,c,hw]
    norm[k,c] = sqrt(sum_hw CR[k,c,hw]^2)
    w[.,c]    = softmax_k(2 * dot[.,c] / norm[.,c])
    out[c,hw] = sum_k IR[k,c,hw] * w[k,c]

Sharding: data-parallel over batch B=8 across the 8 NeuronCores (one b per
core, no cross-core communication).

The kernel is HBM-bound, so inputs are downcast to fp16 on the host before
upload: per-core traffic drops from 72 MB to 36 MB (IR 16 + CR 16 + CT 2 +
OUT 2), which halves the DMA-roofline floor.  The accuracy budget allows it
(fp16 keeps the final error ~8e-4 vs the 2e-2 gate).

Per-core layout: C=256 tiled as 2 x 128 SBUF partitions, HW=4096 on the free
axis.  Engine split, everything paced by the [CT, CR, IR] DMA stream:

- Phase 1 (per (k, c-tile), over the CR stream): ACT does ||CR||^2 via
  Square-with-accumulate (1 elem/cycle/lane); DVE does the dot in a single
  fused pass via scalar_tensor_tensor (elementwise mult + free-axis
  accumulate).  Both accumulate fp32.
- Softmax over K=8: tiny [128, 8] fp32 ops.
- Phase 3: all K=8 weighted-sum terms run on TensorE as fp16 matmuls with
  the 128x128 diagonal diag(w[:,k]) accumulating in PSUM (per-partition
  scaling = diagonal matmul), then DVE/ACT copy PSUM->SBUF (fp16) per half.
  All IR dma_starts for both c-tiles are issued before any OUT dma_start:
  the sync queue is FIFO, and an OUT that sem-waits on a PSUM-copy must not
  head-of-line-block the t=1 IR stream.
"""

import os
import sys

import numpy as np


def _import_concourse():
    try:
        import concourse.bass  # noqa: F401
    except ImportError:
        for p in ("/opt/trn_rl_repo", "/root/.axon_site/_ro/trn_rl_repo"):
            if os.path.isdir(p) and p not in sys.path:
                sys.path.insert(0, p)
        import concourse.bass  # noqa: F401


_import_concourse()

import concourse.bass as bass  # noqa: E402
import concourse.mybir as mybir  # noqa: E402
from concourse import tile  # noqa: E402
from concourse.bass_utils import run_bass_kernel_spmd  # noqa: E402
from concourse.vector_clock import ScopedClock, VectorClock  # noqa: E402


def _split_multiwait_bir(bir: bytes) -> bytes:
    """The neuronxcc walrus in this container encodes at most ONE sync-wait
    per instruction; Tile emits several.  Hoist extra waits onto same-engine
    NoOp instructions inserted immediately before the original instruction
    (engines execute in order, so waiting earlier on the same engine is
    semantically identical).  Sem *updates* are left untouched (a DMA's
    completion-inc cannot move to a sequencer NoOp)."""
    import json

    j = json.loads(bir)
    ctr = 0
    for f in j.get("functions", []):
        for bb in f.get("blocks", []):
            out_insts = []
            for ins in bb.get("instructions", []):
                si = ins.get("sync_info")
                waits = (si or {}).get("on_wait") or []
                if len(waits) > 1:
                    for w in waits[:-1]:
                        ctr += 1
                        nop = {
                            "engine": ins["engine"],
                            "ins": [],
                            "outs": [],
                            "name": f"waitsplit-{ctr}",
                            "opcode": "NoOp",
                            "sync_info": {"on_update": [], "on_wait": [w]},
                        }
                        if "debug" in ins:
                            nop["debug"] = ins["debug"]
                        out_insts.append(nop)
                    si["on_wait"] = [waits[-1]]
                out_insts.append(ins)
            bb["instructions"] = out_insts
    return json.dumps(j).encode()


_orig_to_json_bytes = bass.Bass.to_json_bytes


def _patched_to_json_bytes(self, *a, **kw):
    return _split_multiwait_bir(_orig_to_json_bytes(self, *a, **kw))


bass.Bass.to_json_bytes = _patched_to_json_bytes


def _patched_drain_and_barrier(self, tick_clock, wait_clock):
    # Stock TileContext exit emits one Drain waiting on every used semaphore,
    # which this walrus rejects ("Too many sync wait commands").  Emit one
    # Drain per semaphore instead.
    gc = tick_clock.global_clock
    n = len(gc)
    nonzero = [p for p in range(n) if gc[p] > 0] or [0]
    for p in nonzero:
        d = self.nc.sync.drain()
        vec = [gc[q] if q == p else 0 for q in range(n)]
        wait_clock.add_sem_waits(d.ins, ScopedClock({None: VectorClock(vec)}))
    self.nc.all_engine_barrier()
    popped = self.nc._tile_sem_poison_stack.pop()
    assert popped is self._sem_poison
    self.nc.clear_and_free_semaphores(list(self.sems.allocated().values()))
    self.nc.all_engine_barrier()


tile.TileContext._drain_and_barrier = _patched_drain_and_barrier

FP = mybir.dt.float32
F16 = mybir.dt.float16
B, K, C, H, W = 8, 8, 256, 64, 64
HW = H * W
P = 128                 # SBUF partitions
NCT = C // P            # 2 c-tiles per core
MMN = 512               # moving free dim per matmul (= one PSUM bank of f32)
NMM = HW // MMN         # 8 matmuls per (k, c-tile)

_AF = mybir.ActivationFunctionType
_OP = mybir.AluOpType
_X = mybir.AxisListType.X


def build_nc() -> bass.Bass:
    nc = bass.Bass()
    IR = nc.declare_dram_parameter("IR", [K, C, HW], F16, isOutput=False)
    CR = nc.declare_dram_parameter("CR", [K, C, HW], F16, isOutput=False)
    CT = nc.declare_dram_parameter("CT", [C, HW], F16, isOutput=False)
    OUT = nc.declare_dram_parameter("OUT", [C, HW], F16, isOutput=True)

    with tile.TileContext(nc) as tc:
        with (
            tc.tile_pool(name="ctp", bufs=1) as ct_pool,
            tc.tile_pool(name="crp", bufs=6) as cr_pool,
            tc.tile_pool(name="irp", bufs=8) as ir_pool,
            tc.tile_pool(name="scr", bufs=2) as scr_pool,
            tc.tile_pool(name="sml", bufs=1) as small,
            tc.tile_pool(name="wkp", bufs=2) as wk_pool,
            tc.tile_pool(name="obp", bufs=2) as out_pool,
            tc.tile_pool(name="psp", bufs=1, space="PSUM") as psum_pool,
        ):
            acc = psum_pool.tile([P, HW], FP, name="acc")
            # ACT's elementwise outputs in phase 1 are dead; sink them into an
            # out-pool slot (idle until phase 3, same tag -> shared slots).
            sink = out_pool.tile([P, HW], F16, name="sink", tag="ob")

            # Diagonal ones mask, built once: mask[p, f] = (p == f).
            ones_t = small.tile([P, P], F16, name="ones_t")
            nc.vector.memset(ones_t[:], 1.0)
            mask = small.tile([P, P], F16, name="mask")
            nc.gpsimd.affine_select(
                mask[:],
                ones_t[:],
                pattern=[[-1, P]],
                compare_op=_OP.is_equal,
                fill=0.0,
                base=0,
                channel_multiplier=1,
            )

            # Content-target features stay resident in SBUF (reused by all k).
            ct_tiles = []
            for t in range(NCT):
                ctt = ct_pool.tile([P, HW], F16, name=f"ct{t}", tag=f"ct{t}")
                nc.sync.dma_start(out=ctt[:], in_=CT[t * P:(t + 1) * P, :])
                ct_tiles.append(ctt)

            # ---- Phase 1: dot[c,k] and sq[c,k] reductions over HW ----
            dots = []
            sqs = []
            for t in range(NCT):
                cs = slice(t * P, (t + 1) * P)
                dot = small.tile([P, K], FP, name=f"dot{t}", tag=f"dot{t}")
                sq = small.tile([P, K], FP, name=f"sq{t}", tag=f"sq{t}")
                for k in range(K):
                    crt = cr_pool.tile([P, HW], F16, name="crt", tag="cr")
                    nc.sync.dma_start(out=crt[:], in_=CR[k, cs, :])
                    # ||CR||^2 on ACT (Square-with-accumulate, fp32 accum).
                    nc.scalar.activation(
                        out=sink[:], in_=crt[:], func=_AF.Square,
                        accum_out=sq[:, k:k + 1],
                    )
                    # dot on DVE: fused mult + free-axis accumulate (fp32)
                    # via scalar_tensor_tensor (the only fused-reduce DVE op
                    # this walrus can encode).
                    prod = scr_pool.tile([P, HW], F16, name="prod", tag="scr")
                    nc.vector.scalar_tensor_tensor(
                        out=prod[:],
                        in0=ct_tiles[t][:],
                        scalar=1.0,
                        in1=crt[:],
                        op0=_OP.bypass,
                        op1=_OP.mult,
                        accum_out=dot[:, k:k + 1],
                    )
                dots.append(dot)
                sqs.append(sq)

            # ---- Softmax weights (tiny [128, K] fp32 ops) ----
            ws = []
            for t in range(NCT):
                dot, sq = dots[t], sqs[t]
                norm = small.tile([P, K], FP, name=f"norm{t}", tag=f"norm{t}")
                nc.scalar.activation(norm[:], sq[:], func=_AF.Sqrt)
                rnorm = small.tile([P, K], FP, name=f"rnorm{t}", tag=f"rnorm{t}")
                nc.vector.reciprocal(rnorm[:], norm[:])
                sim = small.tile([P, K], FP, name=f"sim{t}", tag=f"sim{t}")
                nc.vector.tensor_mul(sim[:], dot[:], rnorm[:])
                mx = small.tile([P, 1], FP, name=f"mx{t}", tag=f"mx{t}")
                nc.vector.reduce_max(mx[:], sim[:], axis=_X)
                nbias = small.tile([P, 1], FP, name=f"nb{t}", tag=f"nb{t}")
                nc.vector.tensor_scalar_mul(nbias[:], mx[:], -2.0)
                e = small.tile([P, K], FP, name=f"e{t}", tag=f"e{t}")
                nc.scalar.activation(
                    e[:], sim[:], func=_AF.Exp, bias=nbias[:, 0:1], scale=2.0
                )
                s = small.tile([P, 1], FP, name=f"s{t}", tag=f"s{t}")
                nc.vector.reduce_sum(s[:], e[:], axis=_X)
                rs = small.tile([P, 1], FP, name=f"rs{t}", tag=f"rs{t}")
                nc.vector.reciprocal(rs[:], s[:])
                w = small.tile([P, K], FP, name=f"w{t}", tag=f"w{t}")
                nc.vector.tensor_scalar_mul(w[:], e[:], rs[:, 0:1])
                ws.append(w)

            # ---- Phase 3: out[c,:] = sum_k IR[k,c,:] * w[c,k], all on PE ----
            # All IR dma_starts for both c-tiles are issued before any OUT
            # dma_start: the sync queue is FIFO, and an OUT that sem-waits on
            # a PSUM-copy must not head-of-line-block the t=1 IR stream.
            HH = HW // 2
            obs = []
            for t in range(NCT):
                cs = slice(t * P, (t + 1) * P)
                w = ws[t]
                for k in range(K):
                    wm = wk_pool.tile([P, P], F16, name="wm", tag="wm")
                    nc.vector.tensor_scalar_mul(wm[:], mask[:], w[:, k:k + 1])
                    irt = ir_pool.tile([P, HW], F16, name="irt", tag="ir")
                    nc.sync.dma_start(out=irt[:], in_=IR[k, cs, :])
                    for j in range(NMM):
                        col = j * MMN
                        nc.tensor.matmul(
                            acc[:, col:col + MMN],
                            wm[:],
                            irt[:, col:col + MMN],
                            start=(k == 0),
                            stop=(k == K - 1),
                        )
                # PSUM -> SBUF (fp32 -> fp16) per half as soon as each half's
                # accumulation group stops: h=0 on DVE, h=1 on ACT (parallel
                # engines shorten the drain on the critical tail).  t=1's
                # start-matmuls wait for these reads (PSUM WAR), but the PE
                # has slack.
                ob = out_pool.tile([P, HW], F16, name="ob", tag="ob")
                for h in range(2):
                    hs = slice(h * HH, (h + 1) * HH)
                    nc.vector.tensor_scalar_mul(ob[:, hs], acc[:, hs], 1.0)
                obs.append(ob)
            # OUT dma_starts issued last on the sync queue (see above).
            for t in range(NCT):
                cs = slice(t * P, (t + 1) * P)
                for h in range(2):
                    hs = slice(h * HH, (h + 1) * HH)
                    nc.sync.dma_start(out=OUT[cs, hs], in_=obs[t][:, hs])

    return nc


_NC_CACHE = None


def _get_nc() -> bass.Bass:
    global _NC_CACHE
    if _NC_CACHE is None:
        _NC_CACHE = build_nc()
    return _NC_CACHE


def run(inputs: dict, trace: bool = False):
    """Shard over B, run on 8 cores, gather. Returns (output, BassKernelResults)."""
    ir = np.asarray(inputs["IR_features"], dtype=np.float32)
    cr = np.asarray(inputs["CR_features"], dtype=np.float32)
    ct = np.asarray(inputs["CT_feature"], dtype=np.float32)
    assert ir.shape == (B, K, C, H, W) and cr.shape == (B, K, C, H, W)
    assert ct.shape == (B, C, H, W)
    ir16 = np.ascontiguousarray(ir.reshape(B, K, C, HW).astype(np.float16))
    cr16 = np.ascontiguousarray(cr.reshape(B, K, C, HW).astype(np.float16))
    ct16 = np.ascontiguousarray(ct.reshape(B, C, HW).astype(np.float16))

    in_maps = [
        {"IR": ir16[b], "CR": cr16[b], "CT": ct16[b]}
        for b in range(B)
    ]
    res = run_bass_kernel_spmd(_get_nc(), in_maps, list(range(B)), trace=trace)
    out = np.stack([res.results[b]["OUT"].astype(np.float32) for b in range(B)])
    return out.reshape(B, C, H, W), res


def kernel(**inputs) -> np.ndarray:
    return run(inputs)[0]


# revision 20
# speedup vs baseline: 1.0414x; 1.0414x over previous
"""Trainium2 Bass kernel for the DFS-Mixer style-attention module.

Computation (per batch b):
    dot[k,c]  = sum_hw CT[c,hw] * CR[k,c,hw]
    norm[k,c] = sqrt(sum_hw CR[k,c,hw]^2)
    w[.,c]    = softmax_k(2 * dot[.,c] / norm[.,c])
    out[c,hw] = sum_k IR[k,c,hw] * w[k,c]

Sharding: data-parallel over batch B=8 across the 8 NeuronCores (one b per
core, no cross-core communication).

The kernel is HBM-bound, so inputs are downcast to fp16 on the host before
upload: per-core traffic drops from 72 MB to 36 MB (IR 16 + CR 16 + CT 2 +
OUT 2), which halves the DMA-roofline floor.  The accuracy budget allows it
(fp16 keeps the final error ~8e-4 vs the 2e-2 gate).

Per-core layout: C=256 tiled as 2 x 128 SBUF partitions, HW=4096 on the free
axis.  Engine split, everything paced by the [CT, CR, IR] DMA stream:

- Phase 1 (per (k, c-tile), over the CR stream): ACT does ||CR||^2 via
  Square-with-accumulate (1 elem/cycle/lane); DVE does the dot in a single
  fused pass via scalar_tensor_tensor (elementwise mult + free-axis
  accumulate).  Both accumulate fp32.
- Softmax over K=8: tiny [128, 8] fp32 ops.
- Phase 3: all K=8 weighted-sum terms run on TensorE as fp16 matmuls with
  the 128x128 diagonal diag(w[:,k]) accumulating in PSUM (per-partition
  scaling = diagonal matmul), then DVE/ACT copy PSUM->SBUF (fp16) per half.
  All IR dma_starts for both c-tiles are issued before any OUT dma_start:
  the sync queue is FIFO, and an OUT that sem-waits on a PSUM-copy must not
  head-of-line-block the t=1 IR stream.
"""

import os
import sys

import numpy as np


def _import_concourse():
    try:
        import concourse.bass  # noqa: F401
    except ImportError:
        for p in ("/opt/trn_rl_repo", "/root/.axon_site/_ro/trn_rl_repo"):
            if os.path.isdir(p) and p not in sys.path:
                sys.path.insert(0, p)
        import concourse.bass  # noqa: F401


_import_concourse()

import concourse.bass as bass  # noqa: E402
import concourse.mybir as mybir  # noqa: E402
from concourse import tile  # noqa: E402
from concourse.bass_utils import run_bass_kernel_spmd  # noqa: E402
from concourse.vector_clock import ScopedClock, VectorClock  # noqa: E402


def _split_multiwait_bir(bir: bytes) -> bytes:
    """The neuronxcc walrus in this container encodes at most ONE sync-wait
    per instruction; Tile emits several.  Hoist extra waits onto same-engine
    NoOp instructions inserted immediately before the original instruction
    (engines execute in order, so waiting earlier on the same engine is
    semantically identical).  Sem *updates* are left untouched (a DMA's
    completion-inc cannot move to a sequencer NoOp)."""
    import json

    j = json.loads(bir)
    ctr = 0
    for f in j.get("functions", []):
        for bb in f.get("blocks", []):
            out_insts = []
            for ins in bb.get("instructions", []):
                si = ins.get("sync_info")
                waits = (si or {}).get("on_wait") or []
                if len(waits) > 1:
                    for w in waits[:-1]:
                        ctr += 1
                        nop = {
                            "engine": ins["engine"],
                            "ins": [],
                            "outs": [],
                            "name": f"waitsplit-{ctr}",
                            "opcode": "NoOp",
                            "sync_info": {"on_update": [], "on_wait": [w]},
                        }
                        if "debug" in ins:
                            nop["debug"] = ins["debug"]
                        out_insts.append(nop)
                    si["on_wait"] = [waits[-1]]
                out_insts.append(ins)
            bb["instructions"] = out_insts
    return json.dumps(j).encode()


_orig_to_json_bytes = bass.Bass.to_json_bytes


def _patched_to_json_bytes(self, *a, **kw):
    return _split_multiwait_bir(_orig_to_json_bytes(self, *a, **kw))


bass.Bass.to_json_bytes = _patched_to_json_bytes


def _patched_drain_and_barrier(self, tick_clock, wait_clock):
    # Stock TileContext exit emits one Drain waiting on every used semaphore,
    # which this walrus rejects ("Too many sync wait commands").  Emit one
    # Drain per semaphore instead.
    gc = tick_clock.global_clock
    n = len(gc)
    nonzero = [p for p in range(n) if gc[p] > 0] or [0]
    for p in nonzero:
        d = self.nc.sync.drain()
        vec = [gc[q] if q == p else 0 for q in range(n)]
        wait_clock.add_sem_waits(d.ins, ScopedClock({None: VectorClock(vec)}))
    self.nc.all_engine_barrier()
    popped = self.nc._tile_sem_poison_stack.pop()
    assert popped is self._sem_poison
    self.nc.clear_and_free_semaphores(list(self.sems.allocated().values()))
    self.nc.all_engine_barrier()


tile.TileContext._drain_and_barrier = _patched_drain_and_barrier

FP = mybir.dt.float32
F16 = mybir.dt.float16
B, K, C, H, W = 8, 8, 256, 64, 64
HW = H * W
P = 128                 # SBUF partitions
NCT = C // P            # 2 c-tiles per core
MMN = 512               # moving free dim per matmul (= one PSUM bank of f32)
NMM = HW // MMN         # 8 matmuls per (k, c-tile)

_AF = mybir.ActivationFunctionType
_OP = mybir.AluOpType
_X = mybir.AxisListType.X


def build_nc() -> bass.Bass:
    nc = bass.Bass()
    IR = nc.declare_dram_parameter("IR", [K, C, HW], F16, isOutput=False)
    CR = nc.declare_dram_parameter("CR", [K, C, HW], F16, isOutput=False)
    CT = nc.declare_dram_parameter("CT", [C, HW], F16, isOutput=False)
    OUT = nc.declare_dram_parameter("OUT", [C, HW], F16, isOutput=True)

    with tile.TileContext(nc) as tc:
        with (
            tc.tile_pool(name="ctp", bufs=1) as ct_pool,
            tc.tile_pool(name="crp", bufs=6) as cr_pool,
            tc.tile_pool(name="irp", bufs=8) as ir_pool,
            tc.tile_pool(name="scr", bufs=2) as scr_pool,
            tc.tile_pool(name="sml", bufs=1) as small,
            tc.tile_pool(name="wkp", bufs=2) as wk_pool,
            tc.tile_pool(name="obp", bufs=2) as out_pool,
            tc.tile_pool(name="psp", bufs=1, space="PSUM") as psum_pool,
        ):
            acc = psum_pool.tile([P, HW], FP, name="acc")
            # ACT's elementwise outputs in phase 1 are dead; sink them into an
            # out-pool slot (idle until phase 3, same tag -> shared slots).
            sink = out_pool.tile([P, HW], F16, name="sink", tag="ob")

            # Diagonal ones mask, built once: mask[p, f] = (p == f).
            ones_t = small.tile([P, P], F16, name="ones_t")
            nc.vector.memset(ones_t[:], 1.0)
            mask = small.tile([P, P], F16, name="mask")
            nc.gpsimd.affine_select(
                mask[:],
                ones_t[:],
                pattern=[[-1, P]],
                compare_op=_OP.is_equal,
                fill=0.0,
                base=0,
                channel_multiplier=1,
            )

            # Content-target features stay resident in SBUF (reused by all k).
            ct_tiles = [
                ct_pool.tile([P, HW], F16, name=f"ct{t}", tag=f"ct{t}")
                for t in range(NCT)
            ]
            # ct1 is not consumed until t=1's reductions (~55us in); defer its
            # dma so the first CR tile (which gates the whole phase-1
            # pipeline) lands ~3us earlier.
            nc.sync.dma_start(out=ct_tiles[0][:], in_=CT[0:P, :])

            # ---- Phase 1: dot[c,k] and sq[c,k] reductions over HW ----
            dots = []
            sqs = []
            for t in range(NCT):
                cs = slice(t * P, (t + 1) * P)
                dot = small.tile([P, K], FP, name=f"dot{t}", tag=f"dot{t}")
                sq = small.tile([P, K], FP, name=f"sq{t}", tag=f"sq{t}")
                for k in range(K):
                    crt = cr_pool.tile([P, HW], F16, name="crt", tag="cr")
                    nc.sync.dma_start(out=crt[:], in_=CR[k, cs, :])
                    if t == 0 and k == 3:
                        nc.sync.dma_start(
                            out=ct_tiles[1][:], in_=CT[P:2 * P, :]
                        )
                    # ||CR||^2 on ACT (Square-with-accumulate, fp32 accum).
                    nc.scalar.activation(
                        out=sink[:], in_=crt[:], func=_AF.Square,
                        accum_out=sq[:, k:k + 1],
                    )
                    # dot on DVE: fused mult + free-axis accumulate (fp32)
                    # via scalar_tensor_tensor (the only fused-reduce DVE op
                    # this walrus can encode).
                    prod = scr_pool.tile([P, HW], F16, name="prod", tag="scr")
                    nc.vector.scalar_tensor_tensor(
                        out=prod[:],
                        in0=ct_tiles[t][:],
                        scalar=1.0,
                        in1=crt[:],
                        op0=_OP.bypass,
                        op1=_OP.mult,
                        accum_out=dot[:, k:k + 1],
                    )
                dots.append(dot)
                sqs.append(sq)

            # ---- Softmax weights (tiny [128, K] fp32 ops) ----
            ws = []
            for t in range(NCT):
                dot, sq = dots[t], sqs[t]
                norm = small.tile([P, K], FP, name=f"norm{t}", tag=f"norm{t}")
                nc.scalar.activation(norm[:], sq[:], func=_AF.Sqrt)
                rnorm = small.tile([P, K], FP, name=f"rnorm{t}", tag=f"rnorm{t}")
                nc.vector.reciprocal(rnorm[:], norm[:])
                sim = small.tile([P, K], FP, name=f"sim{t}", tag=f"sim{t}")
                nc.vector.tensor_mul(sim[:], dot[:], rnorm[:])
                mx = small.tile([P, 1], FP, name=f"mx{t}", tag=f"mx{t}")
                nc.vector.reduce_max(mx[:], sim[:], axis=_X)
                nbias = small.tile([P, 1], FP, name=f"nb{t}", tag=f"nb{t}")
                nc.vector.tensor_scalar_mul(nbias[:], mx[:], -2.0)
                e = small.tile([P, K], FP, name=f"e{t}", tag=f"e{t}")
                nc.scalar.activation(
                    e[:], sim[:], func=_AF.Exp, bias=nbias[:, 0:1], scale=2.0
                )
                s = small.tile([P, 1], FP, name=f"s{t}", tag=f"s{t}")
                nc.vector.reduce_sum(s[:], e[:], axis=_X)
                rs = small.tile([P, 1], FP, name=f"rs{t}", tag=f"rs{t}")
                nc.vector.reciprocal(rs[:], s[:])
                w = small.tile([P, K], FP, name=f"w{t}", tag=f"w{t}")
                nc.vector.tensor_scalar_mul(w[:], e[:], rs[:, 0:1])
                ws.append(w)

            # ---- Phase 3: out[c,:] = sum_k IR[k,c,:] * w[c,k], all on PE ----
            # All IR dma_starts for both c-tiles are issued before any OUT
            # dma_start: the sync queue is FIFO, and an OUT that sem-waits on
            # a PSUM-copy must not head-of-line-block the t=1 IR stream.
            HH = HW // 2
            QN = HW // 4
            obs = []
            for t in range(NCT):
                cs = slice(t * P, (t + 1) * P)
                w = ws[t]
                for k in range(K):
                    wm = wk_pool.tile([P, P], F16, name="wm", tag="wm")
                    nc.vector.tensor_scalar_mul(wm[:], mask[:], w[:, k:k + 1])
                    irt = ir_pool.tile([P, HW], F16, name="irt", tag="ir")
                    nc.sync.dma_start(out=irt[:], in_=IR[k, cs, :])
                    last = k == K - 1
                    jord = range(NMM - 1, -1, -1) if last else range(NMM)
                    for j in jord:
                        col = j * MMN
                        nc.tensor.matmul(
                            acc[:, col:col + MMN],
                            wm[:],
                            irt[:, col:col + MMN],
                            start=(k == 0),
                            stop=last,
                        )
                # PSUM -> SBUF (fp32 -> fp16) per half as soon as each half's
                # accumulation group stops: h=0 on DVE, h=1 on ACT (parallel
                # engines shorten the drain on the critical tail).  t=1's
                # start-matmuls wait for these reads (PSUM WAR), but the PE
                # has slack.
                # The final k's matmuls run high-bank-first, so quarter
                # q3 stops first: drain quarters q3..q0 on alternating
                # engines (ACT odd, DVE even) — the tail ends on one short
                # copy + one short store instead of a serial 2x half-copy.
                ob = out_pool.tile([P, HW], F16, name="ob", tag="ob")
                for q in (3, 2, 1, 0):
                    qs = slice(q * QN, (q + 1) * QN)
                    if q % 2 == 1:
                        nc.scalar.copy(ob[:, qs], acc[:, qs])
                    else:
                        nc.vector.tensor_scalar_mul(ob[:, qs], acc[:, qs], 1.0)
                obs.append(ob)
            # OUT dma_starts issued last on the sync queue (see above).
            for t in range(NCT):
                cs = slice(t * P, (t + 1) * P)
                for q in (3, 2, 1, 0):
                    qs = slice(q * QN, (q + 1) * QN)
                    nc.sync.dma_start(out=OUT[cs, qs], in_=obs[t][:, qs])

    return nc


_NC_CACHE = None


def _get_nc() -> bass.Bass:
    global _NC_CACHE
    if _NC_CACHE is None:
        _NC_CACHE = build_nc()
    return _NC_CACHE


def run(inputs: dict, trace: bool = False):
    """Shard over B, run on 8 cores, gather. Returns (output, BassKernelResults)."""
    ir = np.asarray(inputs["IR_features"], dtype=np.float32)
    cr = np.asarray(inputs["CR_features"], dtype=np.float32)
    ct = np.asarray(inputs["CT_feature"], dtype=np.float32)
    assert ir.shape == (B, K, C, H, W) and cr.shape == (B, K, C, H, W)
    assert ct.shape == (B, C, H, W)
    ir16 = np.ascontiguousarray(ir.reshape(B, K, C, HW).astype(np.float16))
    cr16 = np.ascontiguousarray(cr.reshape(B, K, C, HW).astype(np.float16))
    ct16 = np.ascontiguousarray(ct.reshape(B, C, HW).astype(np.float16))

    in_maps = [
        {"IR": ir16[b], "CR": cr16[b], "CT": ct16[b]}
        for b in range(B)
    ]
    res = run_bass_kernel_spmd(_get_nc(), in_maps, list(range(B)), trace=trace)
    out = np.stack([res.results[b]["OUT"].astype(np.float32) for b in range(B)])
    return out.reshape(B, C, H, W), res


def kernel(**inputs) -> np.ndarray:
    return run(inputs)[0]
